# revision 30
# baseline (speedup 1.0000x reference)
"""Multi-head attention (B=2, S=2048, D=1024, H=16, causal mask) on 8 trn2
NeuronCores.

Sharding: 2-way data parallel over batch x 4-way tensor parallel over head
groups (4 heads / core).  Core c handles batch c//4, head group c%4.

v2: single fused chunk pipeline.  Projections of query-chunk qc+1 and the
output projection of chunk qc-1 are emitted as "filler" tensor-engine work
between the attention tile steps of chunk qc, so the PE never idles while
the scalar engine runs the softmax exps (the scalar engine is the pacing
resource of the attention inner loop).  Inputs are pre-tiled on the host so
every input DMA is one fat contiguous descriptor per partition.  All biases
that can be folded on the host (bv@wo + bo) are; Q/K biases are added on
the vector engine.  y partials leave straight from PSUM via DMA.

Everything on-chip lives feature-major ("transposed") so no transposes are
ever needed: Q/K projections produce Qh^T/Kh^T [e, t]; scores come out
keys-major [k, q]; exp(p) feeds A@V directly as the moving operand with V
(+ a ones column that makes the softmax denominator fall out of the same
matmul) stationary; the attention output appears as x_att^T [e, q], which
is exactly the layout the output projection wants.  Each core emits its
y^T partial [1024, S] and the host sums the 4 partials of each batch group
(row-parallel TP reduction; on-device collectives are not launchable as
one 8-replica program through this PJRT path).

Softmax runs unnormalized (exp cannot overflow fp16 at these scales), the
two heads of a pair land on different PE row-tiles so their score matmuls
run concurrently, and the denominators of all 4 heads of a chunk are
reciprocated in one batched Ln+Exp pair on the scalar engine.

Mask is handled generically: the [S,S] mask is classified on the host into
128x128 blocks (zero / one / mixed).  Zero blocks are skipped entirely,
mixed blocks get a pattern-multiply after exp with deduplicated patterns
uploaded as data.
"""

import os
import sys

import numpy as np

for _p in ("/opt/trn_rl_repo", "/root/.axon_site/_ro/trn_rl_repo"):
    if os.path.isdir(_p) and _p not in sys.path:
        sys.path.append(_p)

import ml_dtypes  # noqa: E402
from contextlib import ExitStack  # noqa: E402

import concourse.bass as bass  # noqa: E402
import concourse.tile as tile  # noqa: E402
from concourse import mybir  # noqa: E402

# ----- problem constants (hardcoded per contract) ---------------------------
B, S, D, H, DK = 2, 2048, 1024, 16, 64
NCORES = 8
TP = 4                      # head-parallel ways (per batch group)
EL = D // TP                # 256 local head dims = 4 heads
HL = H // TP                # 4 local heads
QC = 512                    # query-chunk (columns per attention pass)
NQC = S // QC               # 4
KT = 128                    # key tile (contraction tile for A@V)
NKT = S // KT               # 16
P = 128
NMT = D // P                # 8 output-feature tiles
SCALE = 1.0 / np.sqrt(DK)

F32 = mybir.dt.float32
F32R = mybir.dt.float32r
F16 = mybir.dt.float16
F16NP = np.float16
BF16 = mybir.dt.bfloat16
BF16NP = ml_dtypes.bfloat16


# ----- host-side mask analysis ---------------------------------------------
class _KTile:
    __slots__ = ("kt", "s0", "s1", "muls", "first", "last")

    def __init__(self, kt, s0, s1, muls):
        self.kt, self.s0, self.s1, self.muls = kt, s0, s1, muls
        self.first = False
        self.last = False


def _mask_plan(mask2d):
    """mask2d: [S, S] ints, mask2d[q, k] (1 = attend).  Returns
    (plan, patterns) where plan[qc] is a list of _KTile and patterns is a
    f16 array [n_pat, 128, 128] of transposed (k-major) mask blocks."""
    mT = (mask2d != 0).astype(np.float32).T          # [k, q]
    nqt = S // KT
    blk = mT.reshape(NKT, KT, nqt, KT).transpose(0, 2, 1, 3)  # [kt, qt, 128, 128]
    sums = blk.sum(axis=(2, 3))
    patterns = []
    pat_idx = {}

    def pattern_id(kt, qt):
        key = blk[kt, qt].tobytes()
        if key not in pat_idx:
            pat_idx[key] = len(patterns)
            patterns.append(blk[kt, qt].astype(np.float16))
        return pat_idx[key]

    qt_per_qc = QC // KT
    plan = []
    for qc in range(NQC):
        tiles = []
        for kt in range(NKT):
            sub = sums[kt, qc * qt_per_qc:(qc + 1) * qt_per_qc]
            nz = [i for i in range(qt_per_qc) if sub[i] > 0]
            if not nz:
                continue
            s0, s1 = nz[0] * KT, (nz[-1] + 1) * KT
            tiles.append(_KTile(kt, s0, s1, None))
        if not tiles:
            raise ValueError(f"query chunk {qc} has no unmasked keys")
        u0 = min(t.s0 for t in tiles)
        u1 = max(t.s1 for t in tiles)
        tiles[0].s0, tiles[0].s1 = u0, u1
        tiles[0].first = True
        tiles[-1].last = True
        for t in tiles:
            muls = []
            for qt in range(t.s0 // KT, t.s1 // KT):
                full = sums[t.kt, qc * qt_per_qc + qt]
                if full != KT * KT:          # zero or mixed -> needs pattern
                    muls.append((qt, pattern_id(t.kt, qc * qt_per_qc + qt)))
            t.muls = muls
        plan.append(tiles)
    pats = np.stack(patterns) if patterns else np.zeros((1, KT, KT), np.float16)
    return plan, pats


def _merge_ranges(ranges):
    """merge sorted [lo, hi) ranges that touch"""
    out = []
    for lo, hi in ranges:
        if out and out[-1][1] == lo:
            out[-1][1] = hi
        else:
            out.append([lo, hi])
    return out


# ----- TileContext with a codegen-safe exit drain ---------------------------
# The stock kernel-tail drain carries one semaphore wait per engine/queue the
# kernel touched; CoreV3 codegen rejects instructions with more than two
# waits ("Too many sync wait commands").  Split the waits across preceding
# sync-engine nops, two per instruction, so the drain itself needs none.
class _TileContext(tile.TileContext):
    def _drain_and_barrier(self, tick_clock, wait_clock):
        from concourse.vector_clock import ScopedClock
        nc = self.nc
        probe = nc.sync.nop()
        wait_clock.add_sem_waits(
            probe.ins, ScopedClock({None: tick_clock.global_clock}))
        si = probe.ins.sync_info
        waits = list(si.on_wait) if si and si.on_wait else []
        if len(waits) > 1:
            probe.ins.sync_info = mybir.SyncInfo(
                on_wait=waits[:1], on_update=list(si.on_update or []))
            for w in waits[1:]:
                n = nc.sync.nop()
                n.ins.sync_info = mybir.SyncInfo(on_wait=[w], on_update=[])
        nc.sync.drain()
        nc.all_engine_barrier()
        assert self.sems is not None
        popped = nc._tile_sem_poison_stack.pop()
        assert popped is self._sem_poison
        nc.clear_and_free_semaphores(list(self.sems.allocated().values()))
        nc.all_engine_barrier()


# The same wait-count limit applies to ordinary engine instructions under
# this walrus build, so after the program is fully built, hoist all but one
# wait of every instruction onto preceding same-engine no-ops.
def _legalize_waits(nc, limit=1):
    for bb in nc.main_func.blocks:
        insts = list(bb.instructions)
        out = []
        for inst in insts:
            si = inst.sync_info
            waits = list(si.on_wait) if si and si.on_wait else []
            if len(waits) > limit:
                for w in waits[:-limit]:
                    nop = mybir.InstNoOp(
                        name=nc.get_next_instruction_name(), ins=[], outs=[])
                    nop.engine = inst.engine
                    nop.sync_info = mybir.SyncInfo(on_wait=[w], on_update=[])
                    nc.register_instruction(nop, overwrite=True)
                    out.append(nop)
                inst.sync_info = mybir.SyncInfo(
                    on_wait=waits[-limit:],
                    on_update=list(si.on_update or []))
            out.append(inst)
        bb.instructions = out


# ----- the bass program -----------------------------------------------------
def build_program(plan, n_pat):
    nc = bass.Bass(num_devices=NCORES)

    xq4 = nc.dram_tensor("xq4", [P, NQC, 8, QC], F16, kind="ExternalInput")
    xk4 = nc.dram_tensor("xk4", [P, NQC, 8, QC], F16, kind="ExternalInput")
    xv4 = nc.dram_tensor("xv4", [P, NQC, 8, QC], F16, kind="ExternalInput")
    wq_d = nc.dram_tensor("wq_d", [P, 8, EL], F16, kind="ExternalInput")
    wk_d = nc.dram_tensor("wk_d", [P, 8, EL], F16, kind="ExternalInput")
    wv_d = nc.dram_tensor("wv_d", [P, 8, EL], F16, kind="ExternalInput")
    wo_d = nc.dram_tensor("wo_d", [P, 2, D], F16, kind="ExternalInput")
    bq2 = nc.dram_tensor("bq2", [P, 2], F32, kind="ExternalInput")
    bk2 = nc.dram_tensor("bk2", [P, 2], F32, kind="ExternalInput")
    pats = nc.dram_tensor("pats", [P, n_pat, KT], F16, kind="ExternalInput")
    yT = nc.dram_tensor("yT", [D, S], F16, kind="ExternalOutput")

    with ExitStack() as ctx:
        tc = ctx.enter_context(_TileContext(nc))
        singles = ctx.enter_context(tc.tile_pool(name="singles", bufs=1))

        # --- persistent SBUF state ---
        wq_sb = singles.tile([P, 8, EL], F16)
        wk_sb = singles.tile([P, 8, EL], F16)
        wv_sb = singles.tile([P, 8, EL], F16)
        wo_sb = singles.tile([P, 2, D], F16)
        bq_sb = singles.tile([P, 2], F32)
        bk_sb = singles.tile([P, 2], F32)
        pat_sb = singles.tile([P, n_pat, KT], F16)
        ones_sb = singles.tile([P, DK], F32)
        # denominator staging: rows 0 and 32 hold the two heads' d; the
        # rows between stay 1.0 so the contiguous [0:33] Ln/Exp pass over
        # them is finite and ignored.  1/d spans too wide a range for f16
        # (d can be ~e^-3 at q=0 or ~2e6 late), so the reciprocal and its
        # matmul broadcast stay f32.
        stage_sb = singles.tile([P, QC], F32)
        lnd_sb = singles.tile([P, QC], F32)
        rcp_sb = singles.tile([P, QC], F32)
        nc.vector.memset(stage_sb[0:33, :], 1.0)
        # the sync DMA queue fans descriptors out across all 16 DMA engines
        # but processes dma_starts in issue order, so the critical opening
        # chain (wk -> xk0 -> wq -> xq0 -> ...) must be issued first; the
        # x DMAs themselves are issued from emit_x_dma interleaved below.
        nc.gpsimd.dma_start(out=bk_sb[:], in_=bk2[:])
        nc.gpsimd.dma_start(out=bq_sb[:], in_=bq2[:])
        nc.gpsimd.dma_start(out=pat_sb[:], in_=pats[:])
        nc.vector.memset(ones_sb[:], 1.0)

        # touch Exp+Ln early so the activation table set loads during the
        # projection phase instead of stalling the first softmax
        warm = singles.tile([P, 1], F32)
        nc.scalar.activation(out=warm[0:1, :], in_=bq_sb[0:1, 0:1],
                             func=mybir.ActivationFunctionType.Exp)
        nc.scalar.activation(out=warm[0:1, :], in_=warm[0:1, :],
                             func=mybir.ActivationFunctionType.Ln)

        Kt = singles.tile([P, 2, S], F16)     # [e-within-tile, e-tile, t]
        Vaug = singles.tile([P, NKT, HL, DK + 1], F16)  # [t-in-kt, kt, h, e|1]
        nc.vector.memset(Vaug[:, :, :, DK:DK + 1], 1.0)

        # --- pools ---
        xin = ctx.enter_context(tc.tile_pool(name="xin", bufs=6))
        qtp = ctx.enter_context(tc.tile_pool(name="qtp", bufs=2))
        ptp = ctx.enter_context(tc.tile_pool(name="ptp", bufs=4))
        nrm = ctx.enter_context(tc.tile_pool(name="nrm", bufs=2))
        cpp = ctx.enter_context(tc.tile_pool(name="cpp", bufs=6))
        ysp = ctx.enter_context(tc.tile_pool(name="ysp", bufs=3))
        xtp = ctx.enter_context(tc.tile_pool(name="xtp", bufs=2))
        spool = ctx.enter_context(tc.tile_pool(name="spool", bufs=2,
                                               space="PSUM"))
        avy = ctx.enter_context(tc.tile_pool(name="avy", bufs=2, space="PSUM"))
        pp = ctx.enter_context(tc.tile_pool(name="pp", bufs=2, space="PSUM"))

        # --- building blocks -------------------------------------------------
        x_tiles = {}       # (name, qc) -> xin tile

        def emit_x_dma(qc, names=("k", "q", "v")):
            for name in names:
                xdr = {"k": xk4, "q": xq4, "v": xv4}[name]
                t = xin.tile([P, 8, QC], F16, tag="xch", name=f"x_{name}{qc}")
                nc.sync.dma_start(out=t[:], in_=xdr[:, qc])
                x_tiles[(name, qc)] = t

        qt_tiles = {}      # qc -> Qt tile [P, 2, QC]

        def proj_kq_group(name, qc, et):
            """one psum group: 8 matmuls + bias add into Kt / Qt"""
            x_ch = x_tiles[(name, qc)]
            if name == "k":
                w_sb, b_sb = wk_sb, bk_sb
            else:
                w_sb, b_sb = wq_sb, bq_sb
            ps = pp.tile([P, QC], F32, tag="pp", name=f"ps_{name}{qc}{et}")
            for ft in range(8):
                nc.tensor.matmul(
                    ps[:],
                    lhsT=w_sb[:, ft, et * P:(et + 1) * P],
                    rhs=x_ch[:, ft, :],
                    start=(ft == 0), stop=(ft == 7))
            if name == "k":
                nc.vector.tensor_scalar_add(
                    out=Kt[:, et, qc * QC:(qc + 1) * QC], in0=ps[:],
                    scalar1=b_sb[:, et:et + 1])
            else:
                if qc not in qt_tiles:
                    qt_tiles[qc] = qtp.tile([P, 2, QC], F16, tag="qt",
                                            name=f"qt{qc}")
                nc.vector.tensor_scalar_add(
                    out=qt_tiles[qc][:, et, :], in0=ps[:],
                    scalar1=b_sb[:, et:et + 1])

        def proj_v_group(qc, tt):
            """one psum group: 8 matmuls + one strided cast into Vaug"""
            x_ch = x_tiles[("v", qc)]
            ktg = qc * (QC // KT) + tt
            ps = pp.tile([P, QC], F32, tag="pp", name=f"ps_v{ktg}")
            for ft in range(8):
                nc.tensor.matmul(
                    ps[:, 0:EL],
                    lhsT=x_ch[:, ft, tt * P:(tt + 1) * P],
                    rhs=wv_sb[:, ft, :],
                    start=(ft == 0), stop=(ft == 7))
            nc.vector.tensor_copy(
                out=Vaug[:, ktg, :, 0:DK],
                in_=ps[:, 0:EL].rearrange("p (h e) -> p h e", h=HL))

        yTr = yT.rearrange("(a p) t -> p a t", p=P)

        def yproj_group(qc, mt, xTt):
            yp = pp.tile([P, QC], F32, tag="pp", name=f"yp{qc}{mt}")
            for ct in range(2):
                nc.tensor.matmul(
                    yp[:],
                    lhsT=wo_sb[:, ct, mt * P:(mt + 1) * P],
                    rhs=xTt[:, ct, :],
                    start=(ct == 0), stop=(ct == 1))
            ys = ysp.tile([P, QC], F16, tag="ys", name=f"ys{qc}{mt}")
            nc.vector.tensor_copy(out=ys[:], in_=yp[:])
            nc.gpsimd.dma_start(out=yTr[:, mt, qc * QC:(qc + 1) * QC],
                                in_=ys[:])

        # --- the fused pipeline ---------------------------------------------
        def attention(qc, fillers, xTt):
            """attention for chunk qc; pops one filler thunk per tile step"""
            tiles = plan[qc]
            Qt_c = qt_tiles[qc]

            def step_fill():
                if fillers:
                    fillers.pop(0)()

            for hp in range(2):
                et = hp
                avs = [avy.tile([P, QC], F32, tag="avy",
                                name=f"av{qc}{2 * hp + hh}")
                       for hh in range(2)]
                ptbs = [None] * len(tiles)

                def emit_av(ti, t):
                    for hh in range(2):
                        nc.tensor.matmul(
                            avs[hh][0:DK + 1, t.s0:t.s1],
                            lhsT=Vaug[:, t.kt, 2 * hp + hh, :],
                            rhs=ptbs[ti][:, hh, t.s0:t.s1],
                            start=t.first, stop=t.last,
                            skip_group_check=True)

                for ti, t in enumerate(tiles):
                    ps = spool.tile([P, 2 * QC], F32, tag="s",
                                    name=f"s{qc}{hp}{ti}")
                    for hh in range(2):
                        po = hh * DK
                        nc.tensor.matmul(
                            ps[:, hh * QC + t.s0:hh * QC + t.s1],
                            lhsT=Kt[po:po + DK, et,
                                    t.kt * KT:(t.kt + 1) * KT],
                            rhs=Qt_c[po:po + DK, et, t.s0:t.s1],
                            start=True, stop=True)
                    if ti > 0:
                        emit_av(ti - 1, tiles[ti - 1])
                    ptb = ptp.tile([P, 2, QC], F16, tag="pt",
                                   name=f"pt{qc}{hp}{ti}")
                    ptbs[ti] = ptb
                    ptbf = ptb.rearrange("p a c -> p (a c)")
                    rgs = _merge_ranges([(hh * QC + t.s0, hh * QC + t.s1)
                                         for hh in range(2)])
                    for lo, hi in rgs:
                        nc.scalar.activation(
                            out=ptbf[:, lo:hi], in_=ps[:, lo:hi],
                            func=mybir.ActivationFunctionType.Exp,
                            scale=float(SCALE))
                    for hh in range(2):
                        for qt, pid in t.muls:
                            sl = slice(qt * KT, (qt + 1) * KT)
                            nc.vector.tensor_tensor(
                                out=ptb[:, hh, sl],
                                in0=ptb[:, hh, sl],
                                in1=pat_sb[:, pid, :],
                                op=mybir.AluOpType.mult)
                    step_fill()
                emit_av(len(tiles) - 1, tiles[-1])
                # copy the two accumulators out of PSUM (frees the banks)
                hp_cps = []
                for hh in range(2):
                    cp = cpp.tile([P, QC], F32, tag="cp",
                                  name=f"cp{qc}{2 * hp + hh}")
                    nc.vector.tensor_copy(out=cp[0:DK + 1, :],
                                          in_=avs[hh][0:DK + 1, :])
                    hp_cps.append((2 * hp + hh, cp))
                normalize_heads(qc, xTt, hp_cps)

        def normalize_heads(qc, xTt, heads):
            """batched softmax denominators for the given 2 (h, cp) pairs:
            one Ln+Exp reciprocal on the scalar engine (rows at partitions
            0 and 32 -- legal matmul base partitions), then a K=1 matmul
            per head broadcasts 1/d across 64 partitions (all on-chip)."""
            for i, (h, cp) in enumerate(heads):
                nc.gpsimd.dma_start(out=stage_sb[32 * i:32 * i + 1, :],
                                    in_=cp[DK:DK + 1, :])
            nc.scalar.activation(out=lnd_sb[0:33, :], in_=stage_sb[0:33, :],
                                 func=mybir.ActivationFunctionType.Ln)
            nc.scalar.activation(out=rcp_sb[0:33, :], in_=lnd_sb[0:33, :],
                                 func=mybir.ActivationFunctionType.Exp,
                                 scale=-1.0)
            for i, (h, cp) in enumerate(heads):
                et, lo = h // 2, h % 2
                bc = avy.tile([P, QC], F32, tag="avy", name=f"bc{qc}{h}")
                nc.tensor.matmul(bc[0:DK, :],
                                 lhsT=ones_sb[32 * i:32 * i + 1, :],
                                 rhs=rcp_sb[32 * i:32 * i + 1, :],
                                 start=True, stop=True)
                if lo == 0:
                    nc.vector.tensor_tensor(
                        out=xTt[0:DK, et, :], in0=cp[0:DK, :],
                        in1=bc[0:DK, :], op=mybir.AluOpType.mult)
                else:
                    tmp = nrm.tile([P, QC], F16, tag="tmp", name=f"tm{qc}{h}")
                    nc.vector.tensor_tensor(
                        out=tmp[0:DK, :], in0=cp[0:DK, :],
                        in1=bc[0:DK, :], op=mybir.AluOpType.mult)
                    nc.gpsimd.dma_start(out=xTt[DK:P, et, :],
                                        in_=tmp[0:DK, :])

        # startup: the opening DMA chain is ordered so the K projection's
        # inputs land first, then Q's, then V's
        nc.sync.dma_start(out=wk_sb[:], in_=wk_d[:])
        emit_x_dma(0, ("k",))
        nc.sync.dma_start(out=wq_sb[:], in_=wq_d[:])
        emit_x_dma(0, ("q",))
        nc.sync.dma_start(out=wv_sb[:], in_=wv_d[:])
        emit_x_dma(0, ("v",))
        for et in range(2):
            proj_kq_group("k", 0, et)
        for et in range(2):
            proj_kq_group("q", 0, et)
        nc.sync.dma_start(out=wo_sb[:], in_=wo_d[:])
        emit_x_dma(1)
        for tt in range(QC // KT):
            proj_v_group(0, tt)

        xTts = {}
        for qc in range(NQC):
            if qc + 2 < NQC:
                emit_x_dma(qc + 2)
            fillers = []
            if qc + 1 < NQC:
                for et in range(2):
                    fillers.append(
                        lambda q=qc + 1, et=et: proj_kq_group("k", q, et))
                for et in range(2):
                    fillers.append(
                        lambda q=qc + 1, et=et: proj_kq_group("q", q, et))
                for tt in range(QC // KT):
                    fillers.append(
                        lambda q=qc + 1, tt=tt: proj_v_group(q, tt))
            if qc > 0:
                xTt_prev = xTts[qc - 1]
                for mt in range(NMT):
                    fillers.append(
                        lambda q=qc - 1, mt=mt, x=xTt_prev:
                        yproj_group(q, mt, x))
            xTt = xtp.tile([P, 2, QC], F16, tag="xT", name=f"xT{qc}")
            xTts[qc] = xTt
            attention(qc, fillers, xTt)
            for f in fillers:       # leftovers (early chunks)
                f()
        for mt in range(NMT):
            yproj_group(NQC - 1, mt, xTts[NQC - 1])

    _legalize_waits(nc)
    return nc


# ----- SPMD runner ----------------------------------------------------------
# run_bass_kernel_spmd's axon path lowers through jax.jit(shard_map(...)),
# which this jax version emits as `call`-indirect HLO that the bass_exec
# compile hook rejects, and a single 8-replica launch isn't reachable from
# here.  Instead: one single-device jit per core (clean single-computation
# HLO), dispatched asynchronously on all 8 cores.  The NEFF is memoized by
# HLO bytes so walrus runs once, not 8 times.
_NEFF_MEMO = {}


def _install_memo_hook():
    import libneuronxla
    from concourse.bass2jax import install_neuronx_cc_hook

    install_neuronx_cc_hook()
    inner = libneuronxla.neuronx_cc
    if getattr(inner, "_is_memo_hook", False):
        return

    def memo_hook(code, code_format, platform_version, file_prefix):
        import hashlib
        key = hashlib.sha256(bytes(code)).hexdigest()
        if key not in _NEFF_MEMO:
            _NEFF_MEMO[key] = inner(code, code_format, platform_version,
                                    file_prefix)
        return _NEFF_MEMO[key]

    memo_hook._is_memo_hook = True
    libneuronxla.neuronx_cc = memo_hook


def run_spmd(nc, in_maps):
    import jax
    from concourse.bass2jax import _bass_exec_p

    _install_memo_hook()
    n_cores = len(in_maps)
    partition_name = (nc.partition_id_tensor.name
                      if nc.partition_id_tensor is not None else None)
    in_names, out_names, out_avals = [], [], []
    for alloc in nc.m.functions[0].allocations:
        if not isinstance(alloc, mybir.MemoryLocationSet):
            continue
        name = alloc.memorylocations[0].name
        if alloc.kind == "ExternalInput":
            if name != partition_name:
                in_names.append(name)
        elif alloc.kind == "ExternalOutput":
            out_names.append(name)
            out_avals.append(jax.core.ShapedArray(
                tuple(alloc.tensor_shape), mybir.dt.np(alloc.dtype)))
    bind_in_names = tuple(in_names +
                          ([partition_name] if partition_name else []))

    def _body(*args):
        return tuple(_bass_exec_p.bind(
            *args, out_avals=tuple(out_avals), in_names=bind_in_names,
            out_names=tuple(out_names), lowering_input_output_aliases=(),
            sim_require_finite=True, sim_require_nnan=True, nc=nc))

    devices = jax.devices()[:n_cores]
    f = jax.jit(_body)
    futs = []
    for c in range(n_cores):
        args = [jax.device_put(np.asarray(in_maps[c][nm]), devices[c])
                for nm in in_names]
        if partition_name:
            args.append(jax.device_put(np.array([[c]], np.uint32), devices[c]))
        futs.append(f(*args))
    return [{nm: np.asarray(futs[c][i]) for i, nm in enumerate(out_names)}
            for c in range(n_cores)]


# ----- host wrapper ---------------------------------------------------------
_CACHE = {}


def _get_program(mask):
    key = mask.tobytes()
    if key not in _CACHE:
        plan, pats = _mask_plan(mask)
        nc = build_program(plan, pats.shape[0])
        _CACHE[key] = (nc, pats)
    return _CACHE[key]


def _tile_x(xT):
    """[D, S] -> [P, NQC, 8, QC] f16 so each chunk DMA is contiguous"""
    return np.ascontiguousarray(
        xT.reshape(8, P, NQC, QC).transpose(1, 2, 0, 3).astype(F16NP))


def make_in_maps(q, k, v, mask, wq, bq, wk, bk, wv, bv, wo, bo, pats):
    q, k, v = (np.asarray(a, np.float32) for a in (q, k, v))
    pats_t = np.ascontiguousarray(pats.transpose(1, 0, 2))   # [P, n_pat, KT]
    in_maps = []
    for c in range(NCORES):
        b, g = divmod(c, TP)
        sl = slice(g * EL, (g + 1) * EL)
        woT_g = np.ascontiguousarray(wo[:, sl].T)        # [EL, D]
        in_maps.append({
            "xq4": _tile_x(q[b].T),
            "xk4": _tile_x(k[b].T),
            "xv4": _tile_x(v[b].T),
            "wq_d": np.ascontiguousarray(
                wq[sl, :].T.reshape(8, P, EL).transpose(1, 0, 2)
                .astype(F16NP)),
            "wk_d": np.ascontiguousarray(
                wk[sl, :].T.reshape(8, P, EL).transpose(1, 0, 2)
                .astype(F16NP)),
            "wv_d": np.ascontiguousarray(
                wv[sl, :].T.reshape(8, P, EL).transpose(1, 0, 2)
                .astype(F16NP)),
            "wo_d": np.ascontiguousarray(
                woT_g.reshape(2, P, D).transpose(1, 0, 2).astype(F16NP)),
            "bq2": np.ascontiguousarray(bq[sl].reshape(2, P).T),
            "bk2": np.ascontiguousarray(bk[sl].reshape(2, P).T),
            "pats": pats_t,
        })
    return in_maps


def assemble_output(results, bv, wo, bo):
    ybias = (np.asarray(bv, np.float64) @ np.asarray(wo, np.float64).T
             + np.asarray(bo, np.float64)).astype(np.float32)
    y = np.empty((B, S, D), np.float32)
    for b in range(B):
        acc = results[b * TP]["yT"].astype(np.float32)
        for g in range(1, TP):
            acc = acc + results[b * TP + g]["yT"]
        y[b] = acc.T + ybias[None, :]
    return y


def kernel(q, k, v, mask, wq, bq, wk, bk, wv, bv, wo, bo):
    mask2d = np.asarray(mask).reshape(S, S)
    nc, pats = _get_program(mask2d)
    in_maps = make_in_maps(q, k, v, mask2d, wq, bq, wk, bk, wv, bv, wo, bo,
                           pats)
    return assemble_output(run_spmd(nc, in_maps), bv, wo, bo)


# revision 31
# speedup vs baseline: 1.0674x; 1.0674x over previous
"""Multi-head attention (B=2, S=2048, D=1024, H=16, causal mask) on 8 trn2
NeuronCores.

Sharding: 2-way data parallel over batch x 4-way tensor parallel over head
groups (4 heads / core).  Core c handles batch c//4, head group c%4.

v2: single fused chunk pipeline.  Projections of query-chunk qc+1 and the
output projection of chunk qc-1 are emitted as "filler" tensor-engine work
between the attention tile steps of chunk qc, so the PE never idles while
the scalar engine runs the softmax exps (the scalar engine is the pacing
resource of the attention inner loop).  Inputs are pre-tiled on the host so
every input DMA is one fat contiguous descriptor per partition.  All biases
that can be folded on the host (bv@wo + bo) are; Q/K biases are added on
the vector engine.  y partials leave straight from PSUM via DMA.

Everything on-chip lives feature-major ("transposed") so no transposes are
ever needed: Q/K projections produce Qh^T/Kh^T [e, t]; scores come out
keys-major [k, q]; exp(p) feeds A@V directly as the moving operand with V
(+ a ones column that makes the softmax denominator fall out of the same
matmul) stationary; the attention output appears as x_att^T [e, q], which
is exactly the layout the output projection wants.  Each core emits its
y^T partial [1024, S] and the host sums the 4 partials of each batch group
(row-parallel TP reduction; on-device collectives are not launchable as
one 8-replica program through this PJRT path).

Softmax runs unnormalized (exp cannot overflow fp16 at these scales), the
two heads of a pair land on different PE row-tiles so their score matmuls
run concurrently, and the denominators of all 4 heads of a chunk are
reciprocated in one batched Ln+Exp pair on the scalar engine.

Mask is handled generically: the [S,S] mask is classified on the host into
128x128 blocks (zero / one / mixed).  Zero blocks are skipped entirely,
mixed blocks get a pattern-multiply after exp with deduplicated patterns
uploaded as data.
"""

import os
import sys

import numpy as np

for _p in ("/opt/trn_rl_repo", "/root/.axon_site/_ro/trn_rl_repo"):
    if os.path.isdir(_p) and _p not in sys.path:
        sys.path.append(_p)

import ml_dtypes  # noqa: E402
from contextlib import ExitStack  # noqa: E402

import concourse.bass as bass  # noqa: E402
import concourse.tile as tile  # noqa: E402
from concourse import mybir  # noqa: E402

# ----- problem constants (hardcoded per contract) ---------------------------
B, S, D, H, DK = 2, 2048, 1024, 16, 64
NCORES = 8
TP = 4                      # head-parallel ways (per batch group)
EL = D // TP                # 256 local head dims = 4 heads
HL = H // TP                # 4 local heads
QC = 512                    # query-chunk (columns per attention pass)
NQC = S // QC               # 4
KT = 128                    # key tile (contraction tile for A@V)
NKT = S // KT               # 16
P = 128
NMT = D // P                # 8 output-feature tiles
SCALE = 1.0 / np.sqrt(DK)
CRCP = (64.0, 4096.0, 4096.0, 4096.0)   # per-chunk reciprocal scale

F32 = mybir.dt.float32
F32R = mybir.dt.float32r
F16 = mybir.dt.float16
F16NP = np.float16
BF16 = mybir.dt.bfloat16
BF16NP = ml_dtypes.bfloat16


# ----- host-side mask analysis ---------------------------------------------
class _KTile:
    __slots__ = ("kt", "s0", "s1", "muls", "first", "last")

    def __init__(self, kt, s0, s1, muls):
        self.kt, self.s0, self.s1, self.muls = kt, s0, s1, muls
        self.first = False
        self.last = False


def _mask_plan(mask2d):
    """mask2d: [S, S] ints, mask2d[q, k] (1 = attend).  Returns
    (plan, patterns) where plan[qc] is a list of _KTile and patterns is a
    f16 array [n_pat, 128, 128] of transposed (k-major) mask blocks."""
    mT = (mask2d != 0).astype(np.float32).T          # [k, q]
    nqt = S // KT
    blk = mT.reshape(NKT, KT, nqt, KT).transpose(0, 2, 1, 3)  # [kt, qt, 128, 128]
    sums = blk.sum(axis=(2, 3))
    patterns = []
    pat_idx = {}

    def pattern_id(kt, qt):
        key = blk[kt, qt].tobytes()
        if key not in pat_idx:
            pat_idx[key] = len(patterns)
            patterns.append(blk[kt, qt].astype(np.float16))
        return pat_idx[key]

    qt_per_qc = QC // KT
    plan = []
    for qc in range(NQC):
        tiles = []
        for kt in range(NKT):
            sub = sums[kt, qc * qt_per_qc:(qc + 1) * qt_per_qc]
            nz = [i for i in range(qt_per_qc) if sub[i] > 0]
            if not nz:
                continue
            s0, s1 = nz[0] * KT, (nz[-1] + 1) * KT
            tiles.append(_KTile(kt, s0, s1, None))
        if not tiles:
            raise ValueError(f"query chunk {qc} has no unmasked keys")
        u0 = min(t.s0 for t in tiles)
        u1 = max(t.s1 for t in tiles)
        tiles[0].s0, tiles[0].s1 = u0, u1
        tiles[0].first = True
        tiles[-1].last = True
        for t in tiles:
            muls = []
            for qt in range(t.s0 // KT, t.s1 // KT):
                full = sums[t.kt, qc * qt_per_qc + qt]
                if full != KT * KT:          # zero or mixed -> needs pattern
                    muls.append((qt, pattern_id(t.kt, qc * qt_per_qc + qt)))
            t.muls = muls
        plan.append(tiles)
    pats = np.stack(patterns) if patterns else np.zeros((1, KT, KT), np.float16)
    return plan, pats


def _merge_ranges(ranges):
    """merge sorted [lo, hi) ranges that touch"""
    out = []
    for lo, hi in ranges:
        if out and out[-1][1] == lo:
            out[-1][1] = hi
        else:
            out.append([lo, hi])
    return out


# ----- TileContext with a codegen-safe exit drain ---------------------------
# The stock kernel-tail drain carries one semaphore wait per engine/queue the
# kernel touched; CoreV3 codegen rejects instructions with more than two
# waits ("Too many sync wait commands").  Split the waits across preceding
# sync-engine nops, two per instruction, so the drain itself needs none.
class _TileContext(tile.TileContext):
    def _drain_and_barrier(self, tick_clock, wait_clock):
        from concourse.vector_clock import ScopedClock
        nc = self.nc
        probe = nc.sync.nop()
        wait_clock.add_sem_waits(
            probe.ins, ScopedClock({None: tick_clock.global_clock}))
        si = probe.ins.sync_info
        waits = list(si.on_wait) if si and si.on_wait else []
        if len(waits) > 1:
            probe.ins.sync_info = mybir.SyncInfo(
                on_wait=waits[:1], on_update=list(si.on_update or []))
            for w in waits[1:]:
                n = nc.sync.nop()
                n.ins.sync_info = mybir.SyncInfo(on_wait=[w], on_update=[])
        nc.sync.drain()
        nc.all_engine_barrier()
        assert self.sems is not None
        popped = nc._tile_sem_poison_stack.pop()
        assert popped is self._sem_poison
        nc.clear_and_free_semaphores(list(self.sems.allocated().values()))
        nc.all_engine_barrier()


# The same wait-count limit applies to ordinary engine instructions under
# this walrus build, so after the program is fully built, hoist all but one
# wait of every instruction onto preceding same-engine no-ops.
def _legalize_waits(nc, limit=1):
    for bb in nc.main_func.blocks:
        insts = list(bb.instructions)
        out = []
        for inst in insts:
            si = inst.sync_info
            waits = list(si.on_wait) if si and si.on_wait else []
            if len(waits) > limit:
                for w in waits[:-limit]:
                    nop = mybir.InstNoOp(
                        name=nc.get_next_instruction_name(), ins=[], outs=[])
                    nop.engine = inst.engine
                    nop.sync_info = mybir.SyncInfo(on_wait=[w], on_update=[])
                    nc.register_instruction(nop, overwrite=True)
                    out.append(nop)
                inst.sync_info = mybir.SyncInfo(
                    on_wait=waits[-limit:],
                    on_update=list(si.on_update or []))
            out.append(inst)
        bb.instructions = out


# ----- the bass program -----------------------------------------------------
def build_program(plan, n_pat):
    nc = bass.Bass(num_devices=NCORES)

    xq4 = nc.dram_tensor("xq4", [P, NQC, 8, QC], F16, kind="ExternalInput")
    xk4 = nc.dram_tensor("xk4", [P, NQC, 8, QC], F16, kind="ExternalInput")
    xv4 = nc.dram_tensor("xv4", [P, NQC, 8, QC], F16, kind="ExternalInput")
    wq_d = nc.dram_tensor("wq_d", [P, 8, EL], F16, kind="ExternalInput")
    wk_d = nc.dram_tensor("wk_d", [P, 8, EL], F16, kind="ExternalInput")
    wv_d = nc.dram_tensor("wv_d", [P, 8, EL], F16, kind="ExternalInput")
    wo_d = nc.dram_tensor("wo_d", [P, 2, D], F16, kind="ExternalInput")
    bq2 = nc.dram_tensor("bq2", [P, 2], F32, kind="ExternalInput")
    bk2 = nc.dram_tensor("bk2", [P, 2], F32, kind="ExternalInput")
    pats = nc.dram_tensor("pats", [P, n_pat, KT], F16, kind="ExternalInput")
    yT = nc.dram_tensor("yT", [D, S], F16, kind="ExternalOutput")

    with ExitStack() as ctx:
        tc = ctx.enter_context(_TileContext(nc))
        singles = ctx.enter_context(tc.tile_pool(name="singles", bufs=1))

        # --- persistent SBUF state ---
        wq_sb = singles.tile([P, 8, EL], F16)
        wk_sb = singles.tile([P, 8, EL], F16)
        wv_sb = singles.tile([P, 8, EL], F16)
        wo_sb = singles.tile([P, 2, D], F16)
        bq_sb = singles.tile([P, 2], F32)
        bk_sb = singles.tile([P, 2], F32)
        pat_sb = singles.tile([P, n_pat, KT], F16)
        ones_sb = singles.tile([P, DK], F16)
        # denominator staging: rows 0 and 32 hold the two heads' d; the
        # rows between stay 1.0 so the contiguous [0:33] Ln/Exp pass over
        # them is finite and ignored.  The reciprocal is scaled per chunk
        # (c/d with c sized so c/d and c*x_att stay in normal f16 range:
        # chunk 0 rows can have d ~ e^-4, later chunks have d >= ~1) and
        # the y output copy multiplies 1/c back out.
        stage_sb = singles.tile([P, QC], F32)
        lnd_sb = singles.tile([P, QC], F32)
        rcp_sb = singles.tile([P, QC], F16)
        nc.vector.memset(stage_sb[0:33, :], 1.0)
        # the sync DMA queue fans descriptors out across all 16 DMA engines
        # but processes dma_starts in issue order, so the critical opening
        # chain (wk -> xk0 -> wq -> xq0 -> ...) must be issued first; the
        # x DMAs themselves are issued from emit_x_dma interleaved below.
        nc.gpsimd.dma_start(out=bk_sb[:], in_=bk2[:])
        nc.gpsimd.dma_start(out=bq_sb[:], in_=bq2[:])
        nc.gpsimd.dma_start(out=pat_sb[:], in_=pats[:])
        nc.vector.memset(ones_sb[:], 1.0)

        # touch Exp+Ln early so the activation table set loads during the
        # projection phase instead of stalling the first softmax
        warm = singles.tile([P, 1], F32)
        nc.scalar.activation(out=warm[0:1, :], in_=bq_sb[0:1, 0:1],
                             func=mybir.ActivationFunctionType.Exp)
        nc.scalar.activation(out=warm[0:1, :], in_=warm[0:1, :],
                             func=mybir.ActivationFunctionType.Ln)

        Kt = singles.tile([P, 2, S], F16)     # [e-within-tile, e-tile, t]
        Vaug = singles.tile([P, NKT, HL, DK + 1], F16)  # [t-in-kt, kt, h, e|1]
        nc.vector.memset(Vaug[:, :, :, DK:DK + 1], 1.0)

        # --- pools ---
        xin = ctx.enter_context(tc.tile_pool(name="xin", bufs=6))
        qtp = ctx.enter_context(tc.tile_pool(name="qtp", bufs=2))
        ptp = ctx.enter_context(tc.tile_pool(name="ptp", bufs=4))
        nrm = ctx.enter_context(tc.tile_pool(name="nrm", bufs=2))
        cpp = ctx.enter_context(tc.tile_pool(name="cpp", bufs=6))
        ysp = ctx.enter_context(tc.tile_pool(name="ysp", bufs=3))
        xtp = ctx.enter_context(tc.tile_pool(name="xtp", bufs=2))
        spool = ctx.enter_context(tc.tile_pool(name="spool", bufs=2,
                                               space="PSUM"))
        avy = ctx.enter_context(tc.tile_pool(name="avy", bufs=2, space="PSUM"))
        pp = ctx.enter_context(tc.tile_pool(name="pp", bufs=2, space="PSUM"))

        # --- building blocks -------------------------------------------------
        x_tiles = {}       # (name, qc) -> xin tile

        def emit_x_dma(qc, names=("k", "q", "v")):
            for name in names:
                xdr = {"k": xk4, "q": xq4, "v": xv4}[name]
                t = xin.tile([P, 8, QC], F16, tag="xch", name=f"x_{name}{qc}")
                nc.sync.dma_start(out=t[:], in_=xdr[:, qc])
                x_tiles[(name, qc)] = t

        qt_tiles = {}      # qc -> Qt tile [P, 2, QC]

        def proj_kq_group(name, qc, et):
            """one psum group: 8 matmuls + bias add into Kt / Qt"""
            x_ch = x_tiles[(name, qc)]
            if name == "k":
                w_sb, b_sb = wk_sb, bk_sb
            else:
                w_sb, b_sb = wq_sb, bq_sb
            ps = pp.tile([P, QC], F32, tag="pp", name=f"ps_{name}{qc}{et}")
            for ft in range(8):
                nc.tensor.matmul(
                    ps[:],
                    lhsT=w_sb[:, ft, et * P:(et + 1) * P],
                    rhs=x_ch[:, ft, :],
                    start=(ft == 0), stop=(ft == 7))
            if name == "k":
                nc.vector.tensor_scalar_add(
                    out=Kt[:, et, qc * QC:(qc + 1) * QC], in0=ps[:],
                    scalar1=b_sb[:, et:et + 1])
            else:
                if qc not in qt_tiles:
                    qt_tiles[qc] = qtp.tile([P, 2, QC], F16, tag="qt",
                                            name=f"qt{qc}")
                nc.vector.tensor_scalar_add(
                    out=qt_tiles[qc][:, et, :], in0=ps[:],
                    scalar1=b_sb[:, et:et + 1])

        def proj_v_group(qc, tt):
            """one psum group: 8 matmuls + one strided cast into Vaug"""
            x_ch = x_tiles[("v", qc)]
            ktg = qc * (QC // KT) + tt
            ps = pp.tile([P, QC], F32, tag="pp", name=f"ps_v{ktg}")
            for ft in range(8):
                nc.tensor.matmul(
                    ps[:, 0:EL],
                    lhsT=x_ch[:, ft, tt * P:(tt + 1) * P],
                    rhs=wv_sb[:, ft, :],
                    start=(ft == 0), stop=(ft == 7))
            nc.vector.tensor_copy(
                out=Vaug[:, ktg, :, 0:DK],
                in_=ps[:, 0:EL].rearrange("p (h e) -> p h e", h=HL))

        yTr = yT.rearrange("(a p) t -> p a t", p=P)

        def yproj_group(qc, mt, xTt):
            yp = pp.tile([P, QC], F32, tag="pp", name=f"yp{qc}{mt}")
            for ct in range(2):
                nc.tensor.matmul(
                    yp[:],
                    lhsT=wo_sb[:, ct, mt * P:(mt + 1) * P],
                    rhs=xTt[:, ct, :],
                    start=(ct == 0), stop=(ct == 1))
            ys = ysp.tile([P, QC], F16, tag="ys", name=f"ys{qc}{mt}")
            nc.vector.tensor_scalar_mul(out=ys[:], in0=yp[:],
                                        scalar1=1.0 / CRCP[qc])
            nc.gpsimd.dma_start(out=yTr[:, mt, qc * QC:(qc + 1) * QC],
                                in_=ys[:])

        # --- the fused pipeline ---------------------------------------------
        def attention(qc, fillers, xTt):
            """attention for chunk qc; pops one filler thunk per tile step"""
            tiles = plan[qc]
            Qt_c = qt_tiles[qc]

            def step_fill():
                if fillers:
                    fillers.pop(0)()

            for hp in range(2):
                et = hp
                avs = [avy.tile([P, QC], F32, tag="avy",
                                name=f"av{qc}{2 * hp + hh}")
                       for hh in range(2)]
                ptbs = [None] * len(tiles)

                def emit_av(ti, t):
                    for hh in range(2):
                        nc.tensor.matmul(
                            avs[hh][0:DK + 1, t.s0:t.s1],
                            lhsT=Vaug[:, t.kt, 2 * hp + hh, :],
                            rhs=ptbs[ti][:, hh, t.s0:t.s1],
                            start=t.first, stop=t.last,
                            skip_group_check=True)

                for ti, t in enumerate(tiles):
                    ps = spool.tile([P, 2 * QC], F32, tag="s",
                                    name=f"s{qc}{hp}{ti}")
                    for hh in range(2):
                        po = hh * DK
                        nc.tensor.matmul(
                            ps[:, hh * QC + t.s0:hh * QC + t.s1],
                            lhsT=Kt[po:po + DK, et,
                                    t.kt * KT:(t.kt + 1) * KT],
                            rhs=Qt_c[po:po + DK, et, t.s0:t.s1],
                            start=True, stop=True)
                    if ti > 0:
                        emit_av(ti - 1, tiles[ti - 1])
                    ptb = ptp.tile([P, 2, QC], F16, tag="pt",
                                   name=f"pt{qc}{hp}{ti}")
                    ptbs[ti] = ptb
                    ptbf = ptb.rearrange("p a c -> p (a c)")
                    rgs = _merge_ranges([(hh * QC + t.s0, hh * QC + t.s1)
                                         for hh in range(2)])
                    for lo, hi in rgs:
                        nc.scalar.activation(
                            out=ptbf[:, lo:hi], in_=ps[:, lo:hi],
                            func=mybir.ActivationFunctionType.Exp,
                            scale=float(SCALE))
                    for hh in range(2):
                        for qt, pid in t.muls:
                            sl = slice(qt * KT, (qt + 1) * KT)
                            nc.vector.tensor_tensor(
                                out=ptb[:, hh, sl],
                                in0=ptb[:, hh, sl],
                                in1=pat_sb[:, pid, :],
                                op=mybir.AluOpType.mult)
                    step_fill()
                emit_av(len(tiles) - 1, tiles[-1])
                # copy the two accumulators out of PSUM (frees the banks)
                hp_cps = []
                for hh in range(2):
                    cp = cpp.tile([P, QC], F32, tag="cp",
                                  name=f"cp{qc}{2 * hp + hh}")
                    nc.vector.tensor_copy(out=cp[0:DK + 1, :],
                                          in_=avs[hh][0:DK + 1, :])
                    hp_cps.append((2 * hp + hh, cp))
                normalize_heads(qc, xTt, hp_cps)

        def normalize_heads(qc, xTt, heads):
            """batched softmax denominators for the given 2 (h, cp) pairs:
            one Ln+Exp reciprocal on the scalar engine (rows at partitions
            0 and 32 -- legal matmul base partitions), then a K=1 matmul
            per head broadcasts 1/d across 64 partitions (all on-chip)."""
            for i, (h, cp) in enumerate(heads):
                nc.gpsimd.dma_start(out=stage_sb[32 * i:32 * i + 1, :],
                                    in_=cp[DK:DK + 1, :])
            nc.scalar.activation(out=lnd_sb[0:33, :], in_=stage_sb[0:33, :],
                                 func=mybir.ActivationFunctionType.Ln,
                                 scale=1.0 / CRCP[qc])
            nc.scalar.activation(out=rcp_sb[0:33, :], in_=lnd_sb[0:33, :],
                                 func=mybir.ActivationFunctionType.Exp,
                                 scale=-1.0)
            for i, (h, cp) in enumerate(heads):
                et, lo = h // 2, h % 2
                bc = avy.tile([P, QC], F32, tag="avy", name=f"bc{qc}{h}")
                nc.tensor.matmul(bc[0:DK, :],
                                 lhsT=ones_sb[32 * i:32 * i + 1, :],
                                 rhs=rcp_sb[32 * i:32 * i + 1, :],
                                 start=True, stop=True)
                if lo == 0:
                    nc.vector.tensor_tensor(
                        out=xTt[0:DK, et, :], in0=cp[0:DK, :],
                        in1=bc[0:DK, :], op=mybir.AluOpType.mult)
                else:
                    tmp = nrm.tile([P, QC], F16, tag="tmp", name=f"tm{qc}{h}")
                    nc.vector.tensor_tensor(
                        out=tmp[0:DK, :], in0=cp[0:DK, :],
                        in1=bc[0:DK, :], op=mybir.AluOpType.mult)
                    nc.gpsimd.dma_start(out=xTt[DK:P, et, :],
                                        in_=tmp[0:DK, :])

        # startup: the opening DMA chain is ordered so the K projection's
        # inputs land first, then Q's, then V's
        nc.sync.dma_start(out=wk_sb[:], in_=wk_d[:])
        emit_x_dma(0, ("k",))
        nc.sync.dma_start(out=wq_sb[:], in_=wq_d[:])
        emit_x_dma(0, ("q",))
        nc.sync.dma_start(out=wv_sb[:], in_=wv_d[:])
        emit_x_dma(0, ("v",))
        for et in range(2):
            proj_kq_group("k", 0, et)
        for et in range(2):
            proj_kq_group("q", 0, et)
        nc.sync.dma_start(out=wo_sb[:], in_=wo_d[:])
        emit_x_dma(1)
        for tt in range(QC // KT):
            proj_v_group(0, tt)

        xTts = {}
        for qc in range(NQC):
            if qc + 2 < NQC:
                emit_x_dma(qc + 2)
            fillers = []
            if qc + 1 < NQC:
                for et in range(2):
                    fillers.append(
                        lambda q=qc + 1, et=et: proj_kq_group("k", q, et))
                for et in range(2):
                    fillers.append(
                        lambda q=qc + 1, et=et: proj_kq_group("q", q, et))
                for tt in range(QC // KT):
                    fillers.append(
                        lambda q=qc + 1, tt=tt: proj_v_group(q, tt))
            if qc > 0:
                xTt_prev = xTts[qc - 1]
                for mt in range(NMT):
                    fillers.append(
                        lambda q=qc - 1, mt=mt, x=xTt_prev:
                        yproj_group(q, mt, x))
            xTt = xtp.tile([P, 2, QC], F16, tag="xT", name=f"xT{qc}")
            xTts[qc] = xTt
            attention(qc, fillers, xTt)
            for f in fillers:       # leftovers (early chunks)
                f()
        for mt in range(NMT):
            yproj_group(NQC - 1, mt, xTts[NQC - 1])

    _legalize_waits(nc)
    return nc


# ----- SPMD runner ----------------------------------------------------------
# run_bass_kernel_spmd's axon path lowers through jax.jit(shard_map(...)),
# which this jax version emits as `call`-indirect HLO that the bass_exec
# compile hook rejects, and a single 8-replica launch isn't reachable from
# here.  Instead: one single-device jit per core (clean single-computation
# HLO), dispatched asynchronously on all 8 cores.  The NEFF is memoized by
# HLO bytes so walrus runs once, not 8 times.
_NEFF_MEMO = {}


def _install_memo_hook():
    import libneuronxla
    from concourse.bass2jax import install_neuronx_cc_hook

    install_neuronx_cc_hook()
    inner = libneuronxla.neuronx_cc
    if getattr(inner, "_is_memo_hook", False):
        return

    def memo_hook(code, code_format, platform_version, file_prefix):
        import hashlib
        key = hashlib.sha256(bytes(code)).hexdigest()
        if key not in _NEFF_MEMO:
            _NEFF_MEMO[key] = inner(code, code_format, platform_version,
                                    file_prefix)
        return _NEFF_MEMO[key]

    memo_hook._is_memo_hook = True
    libneuronxla.neuronx_cc = memo_hook


def run_spmd(nc, in_maps):
    import jax
    from concourse.bass2jax import _bass_exec_p

    _install_memo_hook()
    n_cores = len(in_maps)
    partition_name = (nc.partition_id_tensor.name
                      if nc.partition_id_tensor is not None else None)
    in_names, out_names, out_avals = [], [], []
    for alloc in nc.m.functions[0].allocations:
        if not isinstance(alloc, mybir.MemoryLocationSet):
            continue
        name = alloc.memorylocations[0].name
        if alloc.kind == "ExternalInput":
            if name != partition_name:
                in_names.append(name)
        elif alloc.kind == "ExternalOutput":
            out_names.append(name)
            out_avals.append(jax.core.ShapedArray(
                tuple(alloc.tensor_shape), mybir.dt.np(alloc.dtype)))
    bind_in_names = tuple(in_names +
                          ([partition_name] if partition_name else []))

    def _body(*args):
        return tuple(_bass_exec_p.bind(
            *args, out_avals=tuple(out_avals), in_names=bind_in_names,
            out_names=tuple(out_names), lowering_input_output_aliases=(),
            sim_require_finite=True, sim_require_nnan=True, nc=nc))

    devices = jax.devices()[:n_cores]
    f = jax.jit(_body)
    futs = []
    for c in range(n_cores):
        args = [jax.device_put(np.asarray(in_maps[c][nm]), devices[c])
                for nm in in_names]
        if partition_name:
            args.append(jax.device_put(np.array([[c]], np.uint32), devices[c]))
        futs.append(f(*args))
    return [{nm: np.asarray(futs[c][i]) for i, nm in enumerate(out_names)}
            for c in range(n_cores)]


# ----- host wrapper ---------------------------------------------------------
_CACHE = {}


def _get_program(mask):
    key = mask.tobytes()
    if key not in _CACHE:
        plan, pats = _mask_plan(mask)
        nc = build_program(plan, pats.shape[0])
        _CACHE[key] = (nc, pats)
    return _CACHE[key]


def _tile_x(xT):
    """[D, S] -> [P, NQC, 8, QC] f16 so each chunk DMA is contiguous"""
    return np.ascontiguousarray(
        xT.reshape(8, P, NQC, QC).transpose(1, 2, 0, 3).astype(F16NP))


def make_in_maps(q, k, v, mask, wq, bq, wk, bk, wv, bv, wo, bo, pats):
    q, k, v = (np.asarray(a, np.float32) for a in (q, k, v))
    pats_t = np.ascontiguousarray(pats.transpose(1, 0, 2))   # [P, n_pat, KT]
    in_maps = []
    for c in range(NCORES):
        b, g = divmod(c, TP)
        sl = slice(g * EL, (g + 1) * EL)
        woT_g = np.ascontiguousarray(wo[:, sl].T)        # [EL, D]
        in_maps.append({
            "xq4": _tile_x(q[b].T),
            "xk4": _tile_x(k[b].T),
            "xv4": _tile_x(v[b].T),
            "wq_d": np.ascontiguousarray(
                wq[sl, :].T.reshape(8, P, EL).transpose(1, 0, 2)
                .astype(F16NP)),
            "wk_d": np.ascontiguousarray(
                wk[sl, :].T.reshape(8, P, EL).transpose(1, 0, 2)
                .astype(F16NP)),
            "wv_d": np.ascontiguousarray(
                wv[sl, :].T.reshape(8, P, EL).transpose(1, 0, 2)
                .astype(F16NP)),
            "wo_d": np.ascontiguousarray(
                woT_g.reshape(2, P, D).transpose(1, 0, 2).astype(F16NP)),
            "bq2": np.ascontiguousarray(bq[sl].reshape(2, P).T),
            "bk2": np.ascontiguousarray(bk[sl].reshape(2, P).T),
            "pats": pats_t,
        })
    return in_maps


def assemble_output(results, bv, wo, bo):
    ybias = (np.asarray(bv, np.float64) @ np.asarray(wo, np.float64).T
             + np.asarray(bo, np.float64)).astype(np.float32)
    y = np.empty((B, S, D), np.float32)
    for b in range(B):
        acc = results[b * TP]["yT"].astype(np.float32)
        for g in range(1, TP):
            acc = acc + results[b * TP + g]["yT"]
        y[b] = acc.T + ybias[None, :]
    return y


def kernel(q, k, v, mask, wq, bq, wk, bk, wv, bv, wo, bo):
    mask2d = np.asarray(mask).reshape(S, S)
    nc, pats = _get_program(mask2d)
    in_maps = make_in_maps(q, k, v, mask2d, wq, bq, wk, bk, wv, bv, wo, bo,
                           pats)
    return assemble_output(run_spmd(nc, in_maps), bv, wo, bo)


# revision 33
# speedup vs baseline: 1.1142x; 1.0438x over previous
"""Multi-head attention (B=2, S=2048, D=1024, H=16, causal mask) on 8 trn2
NeuronCores.

Sharding: 2-way data parallel over batch x 4-way tensor parallel over head
groups (4 heads / core).  Core c handles batch c//4, head group c%4.

v2: single fused chunk pipeline.  Projections of query-chunk qc+1 and the
output projection of chunk qc-1 are emitted as "filler" tensor-engine work
between the attention tile steps of chunk qc, so the PE never idles while
the scalar engine runs the softmax exps (the scalar engine is the pacing
resource of the attention inner loop).  Inputs are pre-tiled on the host so
every input DMA is one fat contiguous descriptor per partition.  All biases
that can be folded on the host (bv@wo + bo) are; Q/K biases are added on
the vector engine.  y partials leave straight from PSUM via DMA.

Everything on-chip lives feature-major ("transposed") so no transposes are
ever needed: Q/K projections produce Qh^T/Kh^T [e, t]; scores come out
keys-major [k, q]; exp(p) feeds A@V directly as the moving operand with V
(+ a ones column that makes the softmax denominator fall out of the same
matmul) stationary; the attention output appears as x_att^T [e, q], which
is exactly the layout the output projection wants.  Each core emits its
y^T partial [1024, S] and the host sums the 4 partials of each batch group
(row-parallel TP reduction; on-device collectives are not launchable as
one 8-replica program through this PJRT path).

Softmax runs unnormalized (exp cannot overflow fp16 at these scales), the
two heads of a pair land on different PE row-tiles so their score matmuls
run concurrently, and the denominators of all 4 heads of a chunk are
reciprocated in one batched Ln+Exp pair on the scalar engine.

Mask is handled generically: the [S,S] mask is classified on the host into
128x128 blocks (zero / one / mixed).  Zero blocks are skipped entirely,
mixed blocks get a pattern-multiply after exp with deduplicated patterns
uploaded as data.
"""

import os
import sys

import numpy as np

for _p in ("/opt/trn_rl_repo", "/root/.axon_site/_ro/trn_rl_repo"):
    if os.path.isdir(_p) and _p not in sys.path:
        sys.path.append(_p)

import ml_dtypes  # noqa: E402
from contextlib import ExitStack  # noqa: E402

import concourse.bass as bass  # noqa: E402
import concourse.tile as tile  # noqa: E402
from concourse import mybir  # noqa: E402

# ----- problem constants (hardcoded per contract) ---------------------------
B, S, D, H, DK = 2, 2048, 1024, 16, 64
NCORES = 8
TP = 4                      # head-parallel ways (per batch group)
EL = D // TP                # 256 local head dims = 4 heads
HL = H // TP                # 4 local heads
QC = 512                    # query-chunk (columns per attention pass)
NQC = S // QC               # 4
KT = 128                    # key tile (contraction tile for A@V)
NKT = S // KT               # 16
P = 128
NMT = D // P                # 8 output-feature tiles
SCALE = 1.0 / np.sqrt(DK)
CRCP = (64.0, 4096.0, 4096.0, 4096.0)   # per-chunk reciprocal scale

F32 = mybir.dt.float32
F32R = mybir.dt.float32r
F16 = mybir.dt.float16
F16NP = np.float16
BF16 = mybir.dt.bfloat16
BF16NP = ml_dtypes.bfloat16


# ----- host-side mask analysis ---------------------------------------------
class _KTile:
    __slots__ = ("kt", "s0", "s1", "muls", "first", "last")

    def __init__(self, kt, s0, s1, muls):
        self.kt, self.s0, self.s1, self.muls = kt, s0, s1, muls
        self.first = False
        self.last = False


def _mask_plan(mask2d):
    """mask2d: [S, S] ints, mask2d[q, k] (1 = attend).  Returns
    (plan, patterns) where plan[qc] is a list of _KTile and patterns is a
    f16 array [n_pat, 128, 128] of transposed (k-major) mask blocks."""
    mT = (mask2d != 0).astype(np.float32).T          # [k, q]
    nqt = S // KT
    blk = mT.reshape(NKT, KT, nqt, KT).transpose(0, 2, 1, 3)  # [kt, qt, 128, 128]
    sums = blk.sum(axis=(2, 3))
    patterns = []
    pat_idx = {}

    def pattern_id(kt, qt):
        key = blk[kt, qt].tobytes()
        if key not in pat_idx:
            pat_idx[key] = len(patterns)
            patterns.append(blk[kt, qt].astype(np.float16))
        return pat_idx[key]

    qt_per_qc = QC // KT
    plan = []
    for qc in range(NQC):
        tiles = []
        for kt in range(NKT):
            sub = sums[kt, qc * qt_per_qc:(qc + 1) * qt_per_qc]
            nz = [i for i in range(qt_per_qc) if sub[i] > 0]
            if not nz:
                continue
            s0, s1 = nz[0] * KT, (nz[-1] + 1) * KT
            tiles.append(_KTile(kt, s0, s1, None))
        if not tiles:
            raise ValueError(f"query chunk {qc} has no unmasked keys")
        u0 = min(t.s0 for t in tiles)
        u1 = max(t.s1 for t in tiles)
        tiles[0].s0, tiles[0].s1 = u0, u1
        tiles[0].first = True
        tiles[-1].last = True
        for t in tiles:
            muls = []
            for qt in range(t.s0 // KT, t.s1 // KT):
                full = sums[t.kt, qc * qt_per_qc + qt]
                if full != KT * KT:          # zero or mixed -> needs pattern
                    muls.append((qt, pattern_id(t.kt, qc * qt_per_qc + qt)))
            t.muls = muls
        plan.append(tiles)
    pats = np.stack(patterns) if patterns else np.zeros((1, KT, KT), np.float16)
    return plan, pats


def _merge_ranges(ranges):
    """merge sorted [lo, hi) ranges that touch"""
    out = []
    for lo, hi in ranges:
        if out and out[-1][1] == lo:
            out[-1][1] = hi
        else:
            out.append([lo, hi])
    return out


# ----- TileContext with a codegen-safe exit drain ---------------------------
# The stock kernel-tail drain carries one semaphore wait per engine/queue the
# kernel touched; CoreV3 codegen rejects instructions with more than two
# waits ("Too many sync wait commands").  Split the waits across preceding
# sync-engine nops, two per instruction, so the drain itself needs none.
class _TileContext(tile.TileContext):
    def _drain_and_barrier(self, tick_clock, wait_clock):
        from concourse.vector_clock import ScopedClock
        nc = self.nc
        probe = nc.sync.nop()
        wait_clock.add_sem_waits(
            probe.ins, ScopedClock({None: tick_clock.global_clock}))
        si = probe.ins.sync_info
        waits = list(si.on_wait) if si and si.on_wait else []
        if len(waits) > 1:
            probe.ins.sync_info = mybir.SyncInfo(
                on_wait=waits[:1], on_update=list(si.on_update or []))
            for w in waits[1:]:
                n = nc.sync.nop()
                n.ins.sync_info = mybir.SyncInfo(on_wait=[w], on_update=[])
        nc.sync.drain()
        nc.all_engine_barrier()
        assert self.sems is not None
        popped = nc._tile_sem_poison_stack.pop()
        assert popped is self._sem_poison
        nc.clear_and_free_semaphores(list(self.sems.allocated().values()))
        nc.all_engine_barrier()


# The same wait-count limit applies to ordinary engine instructions under
# this walrus build, so after the program is fully built, hoist all but one
# wait of every instruction onto preceding same-engine no-ops.
def _legalize_waits(nc, limit=1):
    for bb in nc.main_func.blocks:
        insts = list(bb.instructions)
        out = []
        for inst in insts:
            si = inst.sync_info
            waits = list(si.on_wait) if si and si.on_wait else []
            if len(waits) > limit:
                for w in waits[:-limit]:
                    nop = mybir.InstNoOp(
                        name=nc.get_next_instruction_name(), ins=[], outs=[])
                    nop.engine = inst.engine
                    nop.sync_info = mybir.SyncInfo(on_wait=[w], on_update=[])
                    nc.register_instruction(nop, overwrite=True)
                    out.append(nop)
                inst.sync_info = mybir.SyncInfo(
                    on_wait=waits[-limit:],
                    on_update=list(si.on_update or []))
            out.append(inst)
        bb.instructions = out


# ----- the bass program -----------------------------------------------------
def build_program(plan, n_pat):
    nc = bass.Bass(num_devices=NCORES)

    xq4 = nc.dram_tensor("xq4", [P, NQC, 8, QC], F16, kind="ExternalInput")
    xk4 = nc.dram_tensor("xk4", [P, NQC, 8, QC], F16, kind="ExternalInput")
    xv4 = nc.dram_tensor("xv4", [P, NQC, 8, QC], F16, kind="ExternalInput")
    wq_d = nc.dram_tensor("wq_d", [P, 8, EL], F16, kind="ExternalInput")
    wk_d = nc.dram_tensor("wk_d", [P, 8, EL], F16, kind="ExternalInput")
    wv_d = nc.dram_tensor("wv_d", [P, 8, EL], F16, kind="ExternalInput")
    wo_d = nc.dram_tensor("wo_d", [P, 2, D], F16, kind="ExternalInput")
    bq2 = nc.dram_tensor("bq2", [P, 2], F32, kind="ExternalInput")
    bk2 = nc.dram_tensor("bk2", [P, 2], F32, kind="ExternalInput")
    pats = nc.dram_tensor("pats", [P, n_pat, KT], F16, kind="ExternalInput")
    yT = nc.dram_tensor("yT", [D, S], F16, kind="ExternalOutput")

    with ExitStack() as ctx:
        tc = ctx.enter_context(_TileContext(nc))
        singles = ctx.enter_context(tc.tile_pool(name="singles", bufs=1))

        # --- persistent SBUF state ---
        wq_sb = singles.tile([P, 8, EL], F16)
        wk_sb = singles.tile([P, 8, EL], F16)
        wv_sb = singles.tile([P, 8, EL], F16)
        wo_sb = singles.tile([P, 2, D], F16)
        bq_sb = singles.tile([P, 2], F32)
        bk_sb = singles.tile([P, 2], F32)
        pat_sb = singles.tile([P, n_pat, KT], F16)
        ones_sb = singles.tile([P, DK], F16)
        # denominator staging: rows 0 and 32 hold the two heads' d; the
        # rows between stay 1.0 so the contiguous [0:33] Ln/Exp pass over
        # them is finite and ignored.  The reciprocal is scaled per chunk
        # (c/d with c sized so c/d and c*x_att stay in normal f16 range:
        # chunk 0 rows can have d ~ e^-4, later chunks have d >= ~1) and
        # the y output copy multiplies 1/c back out.
        stage_sb = singles.tile([P, QC], F32)
        lnd_sb = singles.tile([P, QC], F32)
        rcp_sb = singles.tile([P, QC], F16)
        nc.vector.memset(stage_sb[0:33, :], 1.0)
        # the sync DMA queue fans descriptors out across all 16 DMA engines
        # but processes dma_starts in issue order, so the critical opening
        # chain (wk -> xk0 -> wq -> xq0 -> ...) must be issued first; the
        # x DMAs themselves are issued from emit_x_dma interleaved below.
        nc.gpsimd.dma_start(out=bk_sb[:], in_=bk2[:])
        nc.gpsimd.dma_start(out=bq_sb[:], in_=bq2[:])
        nc.gpsimd.dma_start(out=pat_sb[:], in_=pats[:])
        nc.vector.memset(ones_sb[:], 1.0)

        # touch Exp+Ln early so the activation table set loads during the
        # projection phase instead of stalling the first softmax
        warm = singles.tile([P, 1], F32)
        nc.scalar.activation(out=warm[0:1, :], in_=bq_sb[0:1, 0:1],
                             func=mybir.ActivationFunctionType.Exp)
        nc.scalar.activation(out=warm[0:1, :], in_=warm[0:1, :],
                             func=mybir.ActivationFunctionType.Ln)

        Kt = singles.tile([P, 2, S], F16)     # [e-within-tile, e-tile, t]
        Vaug = singles.tile([P, NKT, HL, DK + 1], F16)  # [t-in-kt, kt, h, e|1]
        nc.vector.memset(Vaug[:, :, :, DK:DK + 1], 1.0)

        # --- pools ---
        xin = ctx.enter_context(tc.tile_pool(name="xin", bufs=6))
        qtp = ctx.enter_context(tc.tile_pool(name="qtp", bufs=2))
        ptp = ctx.enter_context(tc.tile_pool(name="ptp", bufs=4))
        nrm = ctx.enter_context(tc.tile_pool(name="nrm", bufs=2))
        cpp = ctx.enter_context(tc.tile_pool(name="cpp", bufs=6))
        ysp = ctx.enter_context(tc.tile_pool(name="ysp", bufs=3))
        xtp = ctx.enter_context(tc.tile_pool(name="xtp", bufs=2))
        spool = ctx.enter_context(tc.tile_pool(name="spool", bufs=2,
                                               space="PSUM"))
        avy = ctx.enter_context(tc.tile_pool(name="avy", bufs=2, space="PSUM"))
        pp = ctx.enter_context(tc.tile_pool(name="pp", bufs=2, space="PSUM"))

        # --- building blocks -------------------------------------------------
        x_tiles = {}       # (name, qc) -> xin tile

        def emit_x_dma(qc, names=("k", "q", "v")):
            for name in names:
                xdr = {"k": xk4, "q": xq4, "v": xv4}[name]
                t = xin.tile([P, 8, QC], F16, tag="xch", name=f"x_{name}{qc}")
                nc.gpsimd.dma_start(out=t[:], in_=xdr[:, qc])
                x_tiles[(name, qc)] = t

        qt_tiles = {}      # qc -> Qt tile [P, 2, QC]

        def proj_kq_group(name, qc, et):
            """one psum group: 8 matmuls + bias add into Kt / Qt"""
            x_ch = x_tiles[(name, qc)]
            if name == "k":
                w_sb, b_sb = wk_sb, bk_sb
            else:
                w_sb, b_sb = wq_sb, bq_sb
            ps = pp.tile([P, QC], F32, tag="pp", name=f"ps_{name}{qc}{et}")
            for ft in range(8):
                nc.tensor.matmul(
                    ps[:],
                    lhsT=w_sb[:, ft, et * P:(et + 1) * P],
                    rhs=x_ch[:, ft, :],
                    start=(ft == 0), stop=(ft == 7))
            if name == "k":
                nc.vector.tensor_scalar_add(
                    out=Kt[:, et, qc * QC:(qc + 1) * QC], in0=ps[:],
                    scalar1=b_sb[:, et:et + 1])
            else:
                if qc not in qt_tiles:
                    qt_tiles[qc] = qtp.tile([P, 2, QC], F16, tag="qt",
                                            name=f"qt{qc}")
                nc.vector.tensor_scalar_add(
                    out=qt_tiles[qc][:, et, :], in0=ps[:],
                    scalar1=b_sb[:, et:et + 1])

        def proj_v_group(qc, tt):
            """one psum group: 8 matmuls + one strided cast into Vaug"""
            x_ch = x_tiles[("v", qc)]
            ktg = qc * (QC // KT) + tt
            ps = pp.tile([P, QC], F32, tag="pp", name=f"ps_v{ktg}")
            for ft in range(8):
                nc.tensor.matmul(
                    ps[:, 0:EL],
                    lhsT=x_ch[:, ft, tt * P:(tt + 1) * P],
                    rhs=wv_sb[:, ft, :],
                    start=(ft == 0), stop=(ft == 7))
            nc.vector.tensor_copy(
                out=Vaug[:, ktg, :, 0:DK],
                in_=ps[:, 0:EL].rearrange("p (h e) -> p h e", h=HL))

        yTr = yT.rearrange("(a p) t -> p a t", p=P)

        def yproj_group(qc, mt, xTt):
            yp = pp.tile([P, QC], F32, tag="pp", name=f"yp{qc}{mt}")
            for ct in range(2):
                nc.tensor.matmul(
                    yp[:],
                    lhsT=wo_sb[:, ct, mt * P:(mt + 1) * P],
                    rhs=xTt[:, ct, :],
                    start=(ct == 0), stop=(ct == 1))
            ys = ysp.tile([P, QC], F16, tag="ys", name=f"ys{qc}{mt}")
            nc.vector.tensor_scalar_mul(out=ys[:], in0=yp[:],
                                        scalar1=1.0 / CRCP[qc])
            nc.sync.dma_start(out=yTr[:, mt, qc * QC:(qc + 1) * QC],
                              in_=ys[:])

        # --- the fused pipeline ---------------------------------------------
        def attention(qc, fillers, xTt):
            """attention for chunk qc; pops one filler thunk per tile step"""
            tiles = plan[qc]
            Qt_c = qt_tiles[qc]

            def step_fill():
                if fillers:
                    fillers.pop(0)()

            for hp in range(2):
                et = hp
                avs = [avy.tile([P, QC], F32, tag="avy",
                                name=f"av{qc}{2 * hp + hh}")
                       for hh in range(2)]
                ptbs = [None] * len(tiles)

                def emit_av(ti, t):
                    for hh in range(2):
                        nc.tensor.matmul(
                            avs[hh][0:DK + 1, t.s0:t.s1],
                            lhsT=Vaug[:, t.kt, 2 * hp + hh, :],
                            rhs=ptbs[ti][:, hh, t.s0:t.s1],
                            start=t.first, stop=t.last,
                            skip_group_check=True)

                for ti, t in enumerate(tiles):
                    ps = spool.tile([P, 2 * QC], F32, tag="s",
                                    name=f"s{qc}{hp}{ti}")
                    for hh in range(2):
                        po = hh * DK
                        nc.tensor.matmul(
                            ps[:, hh * QC + t.s0:hh * QC + t.s1],
                            lhsT=Kt[po:po + DK, et,
                                    t.kt * KT:(t.kt + 1) * KT],
                            rhs=Qt_c[po:po + DK, et, t.s0:t.s1],
                            start=True, stop=True)
                    if ti > 0:
                        emit_av(ti - 1, tiles[ti - 1])
                    ptb = ptp.tile([P, 2, QC], F16, tag="pt",
                                   name=f"pt{qc}{hp}{ti}")
                    ptbs[ti] = ptb
                    ptbf = ptb.rearrange("p a c -> p (a c)")
                    rgs = _merge_ranges([(hh * QC + t.s0, hh * QC + t.s1)
                                         for hh in range(2)])
                    for lo, hi in rgs:
                        nc.scalar.activation(
                            out=ptbf[:, lo:hi], in_=ps[:, lo:hi],
                            func=mybir.ActivationFunctionType.Exp,
                            scale=float(SCALE))
                    for hh in range(2):
                        for qt, pid in t.muls:
                            sl = slice(qt * KT, (qt + 1) * KT)
                            nc.vector.tensor_tensor(
                                out=ptb[:, hh, sl],
                                in0=ptb[:, hh, sl],
                                in1=pat_sb[:, pid, :],
                                op=mybir.AluOpType.mult)
                    step_fill()
                emit_av(len(tiles) - 1, tiles[-1])
                # copy the two accumulators out of PSUM (frees the banks)
                hp_cps = []
                for hh in range(2):
                    cp = cpp.tile([P, QC], F32, tag="cp",
                                  name=f"cp{qc}{2 * hp + hh}")
                    if hh == 0:
                        nc.vector.tensor_copy(out=cp[0:DK + 1, :],
                                              in_=avs[hh][0:DK + 1, :])
                    else:
                        nc.scalar.copy(out=cp[0:DK + 1, :],
                                       in_=avs[hh][0:DK + 1, :])
                    hp_cps.append((2 * hp + hh, cp))
                normalize_heads(qc, xTt, hp_cps)

        def normalize_heads(qc, xTt, heads):
            """batched softmax denominators for the given 2 (h, cp) pairs:
            one Ln+Exp reciprocal on the scalar engine (rows at partitions
            0 and 32 -- legal matmul base partitions), then a K=1 matmul
            per head broadcasts 1/d across 64 partitions (all on-chip)."""
            for i, (h, cp) in enumerate(heads):
                nc.sync.dma_start(out=stage_sb[32 * i:32 * i + 1, :],
                                  in_=cp[DK:DK + 1, :])
            nc.scalar.activation(out=lnd_sb[0:33, :], in_=stage_sb[0:33, :],
                                 func=mybir.ActivationFunctionType.Ln,
                                 scale=1.0 / CRCP[qc])
            nc.scalar.activation(out=rcp_sb[0:33, :], in_=lnd_sb[0:33, :],
                                 func=mybir.ActivationFunctionType.Exp,
                                 scale=-1.0)
            for i, (h, cp) in enumerate(heads):
                et, lo = h // 2, h % 2
                bc = avy.tile([P, QC], F32, tag="avy", name=f"bc{qc}{h}")
                nc.tensor.matmul(bc[0:DK, :],
                                 lhsT=ones_sb[32 * i:32 * i + 1, :],
                                 rhs=rcp_sb[32 * i:32 * i + 1, :],
                                 start=True, stop=True)
                if lo == 0:
                    nc.vector.tensor_tensor(
                        out=xTt[0:DK, et, :], in0=cp[0:DK, :],
                        in1=bc[0:DK, :], op=mybir.AluOpType.mult)
                else:
                    tmp = nrm.tile([P, QC], F16, tag="tmp", name=f"tm{qc}{h}")
                    nc.vector.tensor_tensor(
                        out=tmp[0:DK, :], in0=cp[0:DK, :],
                        in1=bc[0:DK, :], op=mybir.AluOpType.mult)
                    nc.sync.dma_start(out=xTt[DK:P, et, :],
                                       in_=tmp[0:DK, :])

        # startup: the opening DMA chain is ordered so the K projection's
        # inputs land first, then Q's, then V's
        nc.sync.dma_start(out=wk_sb[:], in_=wk_d[:])
        emit_x_dma(0, ("k",))
        nc.sync.dma_start(out=wq_sb[:], in_=wq_d[:])
        emit_x_dma(0, ("q",))
        nc.sync.dma_start(out=wv_sb[:], in_=wv_d[:])
        emit_x_dma(0, ("v",))
        for et in range(2):
            proj_kq_group("k", 0, et)
        for et in range(2):
            proj_kq_group("q", 0, et)
        nc.sync.dma_start(out=wo_sb[:], in_=wo_d[:])
        emit_x_dma(1)
        for tt in range(QC // KT):
            proj_v_group(0, tt)

        xTts = {}
        for qc in range(NQC):
            if qc + 2 < NQC:
                emit_x_dma(qc + 2)
            fillers = []
            if qc > 0:
                # V proj of THIS chunk (deferred from the previous chunk's
                # fillers): tile tt is consumed at attention step 4*qc+tt,
                # and fillers pop one per step, so these always land first
                for tt in range(QC // KT):
                    fillers.append(
                        lambda q=qc, tt=tt: proj_v_group(q, tt))
            if qc + 1 < NQC:
                for et in range(2):
                    fillers.append(
                        lambda q=qc + 1, et=et: proj_kq_group("k", q, et))
                for et in range(2):
                    fillers.append(
                        lambda q=qc + 1, et=et: proj_kq_group("q", q, et))
            if qc > 0:
                xTt_prev = xTts[qc - 1]
                for mt in range(NMT):
                    fillers.append(
                        lambda q=qc - 1, mt=mt, x=xTt_prev:
                        yproj_group(q, mt, x))
            xTt = xtp.tile([P, 2, QC], F16, tag="xT", name=f"xT{qc}")
            xTts[qc] = xTt
            attention(qc, fillers, xTt)
            for f in fillers:       # leftovers (early chunks)
                f()
        for mt in range(NMT):
            yproj_group(NQC - 1, mt, xTts[NQC - 1])

    _legalize_waits(nc)
    return nc


# ----- SPMD runner ----------------------------------------------------------
# run_bass_kernel_spmd's axon path lowers through jax.jit(shard_map(...)),
# which this jax version emits as `call`-indirect HLO that the bass_exec
# compile hook rejects, and a single 8-replica launch isn't reachable from
# here.  Instead: one single-device jit per core (clean single-computation
# HLO), dispatched asynchronously on all 8 cores.  The NEFF is memoized by
# HLO bytes so walrus runs once, not 8 times.
_NEFF_MEMO = {}


def _install_memo_hook():
    import libneuronxla
    from concourse.bass2jax import install_neuronx_cc_hook

    install_neuronx_cc_hook()
    inner = libneuronxla.neuronx_cc
    if getattr(inner, "_is_memo_hook", False):
        return

    def memo_hook(code, code_format, platform_version, file_prefix):
        import hashlib
        key = hashlib.sha256(bytes(code)).hexdigest()
        if key not in _NEFF_MEMO:
            _NEFF_MEMO[key] = inner(code, code_format, platform_version,
                                    file_prefix)
        return _NEFF_MEMO[key]

    memo_hook._is_memo_hook = True
    libneuronxla.neuronx_cc = memo_hook


def run_spmd(nc, in_maps):
    import jax
    from concourse.bass2jax import _bass_exec_p

    _install_memo_hook()
    n_cores = len(in_maps)
    partition_name = (nc.partition_id_tensor.name
                      if nc.partition_id_tensor is not None else None)
    in_names, out_names, out_avals = [], [], []
    for alloc in nc.m.functions[0].allocations:
        if not isinstance(alloc, mybir.MemoryLocationSet):
            continue
        name = alloc.memorylocations[0].name
        if alloc.kind == "ExternalInput":
            if name != partition_name:
                in_names.append(name)
        elif alloc.kind == "ExternalOutput":
            out_names.append(name)
            out_avals.append(jax.core.ShapedArray(
                tuple(alloc.tensor_shape), mybir.dt.np(alloc.dtype)))
    bind_in_names = tuple(in_names +
                          ([partition_name] if partition_name else []))

    def _body(*args):
        return tuple(_bass_exec_p.bind(
            *args, out_avals=tuple(out_avals), in_names=bind_in_names,
            out_names=tuple(out_names), lowering_input_output_aliases=(),
            sim_require_finite=True, sim_require_nnan=True, nc=nc))

    devices = jax.devices()[:n_cores]
    f = jax.jit(_body)
    futs = []
    for c in range(n_cores):
        args = [jax.device_put(np.asarray(in_maps[c][nm]), devices[c])
                for nm in in_names]
        if partition_name:
            args.append(jax.device_put(np.array([[c]], np.uint32), devices[c]))
        futs.append(f(*args))
    return [{nm: np.asarray(futs[c][i]) for i, nm in enumerate(out_names)}
            for c in range(n_cores)]


# ----- host wrapper ---------------------------------------------------------
_CACHE = {}


def _get_program(mask):
    key = mask.tobytes()
    if key not in _CACHE:
        plan, pats = _mask_plan(mask)
        nc = build_program(plan, pats.shape[0])
        _CACHE[key] = (nc, pats)
    return _CACHE[key]


def _tile_x(xT):
    """[D, S] -> [P, NQC, 8, QC] f16 so each chunk DMA is contiguous"""
    return np.ascontiguousarray(
        xT.reshape(8, P, NQC, QC).transpose(1, 2, 0, 3).astype(F16NP))


def make_in_maps(q, k, v, mask, wq, bq, wk, bk, wv, bv, wo, bo, pats):
    q, k, v = (np.asarray(a, np.float32) for a in (q, k, v))
    pats_t = np.ascontiguousarray(pats.transpose(1, 0, 2))   # [P, n_pat, KT]
    in_maps = []
    for c in range(NCORES):
        b, g = divmod(c, TP)
        sl = slice(g * EL, (g + 1) * EL)
        woT_g = np.ascontiguousarray(wo[:, sl].T)        # [EL, D]
        in_maps.append({
            "xq4": _tile_x(q[b].T),
            "xk4": _tile_x(k[b].T),
            "xv4": _tile_x(v[b].T),
            "wq_d": np.ascontiguousarray(
                wq[sl, :].T.reshape(8, P, EL).transpose(1, 0, 2)
                .astype(F16NP)),
            "wk_d": np.ascontiguousarray(
                wk[sl, :].T.reshape(8, P, EL).transpose(1, 0, 2)
                .astype(F16NP)),
            "wv_d": np.ascontiguousarray(
                wv[sl, :].T.reshape(8, P, EL).transpose(1, 0, 2)
                .astype(F16NP)),
            "wo_d": np.ascontiguousarray(
                woT_g.reshape(2, P, D).transpose(1, 0, 2).astype(F16NP)),
            "bq2": np.ascontiguousarray(bq[sl].reshape(2, P).T),
            "bk2": np.ascontiguousarray(bk[sl].reshape(2, P).T),
            "pats": pats_t,
        })
    return in_maps


def assemble_output(results, bv, wo, bo):
    ybias = (np.asarray(bv, np.float64) @ np.asarray(wo, np.float64).T
             + np.asarray(bo, np.float64)).astype(np.float32)
    y = np.empty((B, S, D), np.float32)
    for b in range(B):
        acc = results[b * TP]["yT"].astype(np.float32)
        for g in range(1, TP):
            acc = acc + results[b * TP + g]["yT"]
        y[b] = acc.T + ybias[None, :]
    return y


def kernel(q, k, v, mask, wq, bq, wk, bk, wv, bv, wo, bo):
    mask2d = np.asarray(mask).reshape(S, S)
    nc, pats = _get_program(mask2d)
    in_maps = make_in_maps(q, k, v, mask2d, wq, bq, wk, bk, wv, bv, wo, bo,
                           pats)
    return assemble_output(run_spmd(nc, in_maps), bv, wo, bo)
